# revision 37
# baseline (speedup 1.0000x reference)
"""Trainium2 Bass kernel for nn_Mixture: out = -log(Phi*exp(-0.5*q)/sqrt(2*pi*det(Sigma)) + eps)
with q_n = (x_n - mu)^T pinv(Sigma) (x_n - mu), x: [N, 64] f32.

Strategy (pure data parallel over 8 cores):
  - Host: tiny O(D^3) precompute: pinv/det/eigh of Sigma (64x64), constants.
  - Device per core (N/8 = 131072 samples):
      * contiguous DMA loads [128, 8192] f32 (128 consecutive samples per partition)
      * PE transpose 128x128 chunks -> PSUM (dims land on partitions)
      * ACT: sq = Square(x + (-mu))  (mu folded in as per-partition bias), bf16 out
      * PE: lhsT = block "-0.5*ones" [128, 2] -> QP[2u+g, :] = -0.5 * q  (64->1 reduce)
      * ACT: Softplus(QP + (ln(A) - ln(eps)))   [exact: -log(e^t+eps) = -ln(eps)-softplus(t-ln(eps))]
      * DVE: out = C - softplus(...)
      * PE re-transpose of the small q-grid (interleaved out AP) to restore sample order
      * contiguous DMA out
  If pinv(Sigma) != I: an extra PE matmul with W2 = blockdiag(W, W), W^T W = pinv(Sigma),
  is inserted between transpose and Square (slower but correct for any PSD Sigma).
"""

import math
import os
import sys

import numpy as np

sys.path.insert(0, "/opt/trn_rl_repo")

import concourse.bass as bass  # noqa: E402
import concourse.bacc as bacc_mod  # noqa: E402
import concourse.mybir as mybir  # noqa: E402
from concourse.tile import TileContext  # noqa: E402

TWO_PI = 2.0 * np.pi
EPS = 1e-8

# Force all our ACT funcs (Square/Exp/Ln, + Copy/Identity for the general
# path) into ONE table set so the scalar engine never swaps table sets
# mid-kernel (~2.7us per swap). We keep the set list order identical (ids
# are positional indices into act_info.json) and just remove our funcs
# from every other set so the chooser must pick the shared one.
_ACT_PATCHED = False


def _patch_act_tables():
    global _ACT_PATCHED
    if _ACT_PATCHED:
        return
    import functools

    import concourse.hw_specs as hw_specs
    import concourse.bacc as _bacc

    orig = hw_specs.get_activation_tables
    AF = mybir.ActivationFunctionType
    need = {AF.Square, AF.Exp, AF.Ln, AF.Identity, AF.Copy, AF.MemsetZero}
    keep = "natural_log_exp_and_others"

    @functools.cache
    def patched(module_arch):
        t = dict(orig(module_arch))
        if keep in t and need <= t[keep]:
            t = {
                name: (funcs if name == keep else funcs - need)
                for name, funcs in t.items()
            }
        return t

    hw_specs.get_activation_tables = patched
    _bacc.get_activation_tables = patched
    _ACT_PATCHED = True

P = 128          # partitions
D = 64           # sample dim
CHUNK = 128      # transpose chunk cols
MG = 8           # m-groups per macro tile
TILE_COLS = MG * 512           # 4096 f32 per partition per macro tile
TILE_SAMPLES = P * TILE_COLS // D  # 8192 samples per macro tile
BATCH_TILES = 8  # macro tiles per q-grid batch (grid 128x512)


def _build_nc(n_per_core: int, tiles_per_batch: int, general: bool):
    """Build the SPMD per-core Bass program."""
    _patch_act_tables()
    assert n_per_core % TILE_SAMPLES == 0
    n_tiles = n_per_core // TILE_SAMPLES
    assert n_tiles % tiles_per_batch == 0
    n_batches = n_tiles // tiles_per_batch
    qp_rows = 2 * tiles_per_batch * MG
    assert qp_rows <= 128

    f32 = mybir.dt.float32
    bf16 = mybir.dt.bfloat16
    AF = mybir.ActivationFunctionType

    nc = bacc_mod.Bacc("TRN2")
    x = nc.dram_tensor("x", [n_per_core, D], f32, kind="ExternalInput")
    ident = nc.dram_tensor("ident", [P, P], bf16, kind="ExternalInput")
    negmu2 = nc.dram_tensor("negmu2", [P, 1], f32, kind="ExternalInput")
    bias_tr = nc.dram_tensor("bias_tr", [P, 1], f32, kind="ExternalInput")
    negh2 = nc.dram_tensor("negh2", [P, 2], bf16, kind="ExternalInput")
    if general:
        w2 = nc.dram_tensor("w2", [P, P], bf16, kind="ExternalInput")
    out = nc.dram_tensor("out", [n_per_core], f32, kind="ExternalOutput")

    jw = TILE_COLS // D  # samples per partition per tile (q-grid cols per tl)
    # [tile, 128, TILE_COLS] view: partition p holds jw consecutive samples
    x_v = x.rearrange("(t p k) d -> t p (k d)", p=P, k=jw)
    # output view: sample = b*BS + tl*TILE_SAMPLES + r*jw + j
    out_v = out.rearrange(
        "(b t r j) -> b r t j", t=tiles_per_batch, r=P, j=jw
    )
    grid_cols = tiles_per_batch * jw

    C = -math.log(EPS)

    # m-groups per PSUM tile: 4 (fast; [128,2048] bf16 = 2 banks) keeps the
    # ACT squares at FD=2048; general path needs PSUM room for z
    m2w = 2 if general else 4
    with TileContext(nc) as tc:
        with (
            tc.tile_pool(name="consts", bufs=1) as cpool,
            tc.tile_pool(name="xin", bufs=5) as in_pool,
            tc.tile_pool(name="xinb", bufs=8) as inb_pool,
            tc.tile_pool(name="tp", bufs=2 if general else 3, space="PSUM") as tp_pool,
            tc.tile_pool(name="z", bufs=1, space="PSUM") as z_pool,
            tc.tile_pool(name="sq", bufs=6) as sq_pool,
            tc.tile_pool(name="qp", bufs=2, space="PSUM") as qp_pool,
            tc.tile_pool(name="post", bufs=2) as post_pool,
        ):
            ident_t = cpool.tile([P, P], bf16)
            nc.sync.dma_start(out=ident_t, in_=ident[:, :])
            negmu2_t = cpool.tile([P, 1], f32)
            nc.sync.dma_start(out=negmu2_t, in_=negmu2[:, :])
            bias_tr_t = cpool.tile([P, 1], f32)
            nc.sync.dma_start(out=bias_tr_t, in_=bias_tr[:, :])
            negh2_t = cpool.tile([P, 2], bf16)
            nc.sync.dma_start(out=negh2_t, in_=negh2[:, :])
            eps_t = cpool.tile([P, 1], f32)
            nc.vector.memset(eps_t, float(EPS))
            if general:
                w2_t = cpool.tile([P, P], bf16)
                nc.sync.dma_start(out=w2_t, in_=w2[:, :])

            for b in range(n_batches):
                qp = qp_pool.tile([P, grid_cols], f32, name=f"qp_{b}", tag="qp")
                for tl in range(tiles_per_batch):
                    t = b * tiles_per_batch + tl
                    tin = in_pool.tile([P, TILE_COLS], f32, tag="tin")
                    tinb = inb_pool.tile([P, TILE_COLS], bf16, tag="tinb")
                    # cast to bf16 on the (otherwise idle) vector engine:
                    # halves the PE transpose cost and makes the stationary
                    # loads FWL-eligible. Sliced DMAs + casts pipeline the
                    # first tile so transposes start after ~512KB.
                    if t == 0:
                        n_sl = 4
                        scols = TILE_COLS // n_sl
                        for si in range(n_sl):
                            sl = slice(si * scols, (si + 1) * scols)
                            nc.sync.dma_start(
                                out=tin[:, sl], in_=x_v[t][:, sl]
                            )
                            nc.vector.tensor_copy(tinb[:, sl], tin[:, sl])
                    else:
                        nc.sync.dma_start(out=tin, in_=x_v[t])
                        nc.vector.tensor_copy(tinb, tin)
                    for m2 in range(MG // m2w):
                        # m2w m-groups share one PSUM tile so the ACT
                        # square runs at a large free dim
                        tp = tp_pool.tile([P, 512 * m2w], bf16, tag="tp")
                        for mh in range(m2w):
                            m = m2 * m2w + mh
                            for c in range(4):
                                nc.tensor.transpose(
                                    tp[:, mh * 512 + c * CHUNK:
                                       mh * 512 + (c + 1) * CHUNK],
                                    tinb[:, m * 512 + c * CHUNK:
                                         m * 512 + (c + 1) * CHUNK],
                                    ident_t,
                                )
                        sq = sq_pool.tile([P, 512 * m2w], bf16, tag="sq")
                        if general:
                            # z = W2 @ x, then sq = Square(z - W2@mu2)
                            xt = sq_pool.tile([P, 512 * m2w], bf16, tag="xt")
                            nc.scalar.activation(xt, tp, AF.Copy, scale=1.0)
                            z = z_pool.tile([P, 512 * m2w], f32, tag="z")
                            nc.tensor.matmul(
                                z, w2_t, xt, start=True, stop=True
                            )
                            nc.scalar.activation(
                                sq, z, AF.Square, bias=negmu2_t, scale=1.0
                            )
                        else:
                            nc.scalar.activation(
                                sq, tp, AF.Square, bias=negmu2_t, scale=1.0
                            )
                        # reduce 64 dims -> q: sq chunk is the STATIONARY
                        # (samples land on output partitions), -0.5 block
                        # matrix is the moving; output col-pair lands at
                        # free offset tl*jw + m*8 + c*2 in the q grid.
                        for mh in range(m2w):
                            m = m2 * m2w + mh
                            for c in range(4):
                                base = tl * jw + m * 8 + c * 2
                                nc.tensor.matmul(
                                    qp[:, base:base + 2],
                                    sq[:, mh * 512 + c * CHUNK:
                                       mh * 512 + (c + 1) * CHUNK],
                                    negh2_t,
                                    start=True, stop=True,
                                )
                # post-process the q grid: out = -log(exp(qp + lnA) + eps)
                s = post_pool.tile([P, grid_cols], f32, tag="s")
                nc.scalar.activation(
                    s, qp, AF.Exp, bias=bias_tr_t, scale=1.0,
                )
                s2 = post_pool.tile([P, grid_cols], f32, tag="s2")
                nc.scalar.activation(
                    s2, s, AF.Ln, bias=eps_t, scale=1.0,
                )
                o = post_pool.tile([P, grid_cols], f32, tag="o")
                nc.vector.tensor_scalar(
                    o, s2, -1.0, 0.0, mybir.AluOpType.mult, mybir.AluOpType.add
                )
                nc.sync.dma_start(
                    out=out_v[b],
                    in_=o.rearrange("p (t j) -> p t j", t=tiles_per_batch),
                )
    nc.finalize()
    return nc


def _host_prep(Phi, mu, Sigma):
    """Host-side tiny precompute. Returns (consts dict, general, W2)."""
    Sigma = np.asarray(Sigma, dtype=np.float64)
    mu = np.asarray(mu, dtype=np.float32)
    inv = np.linalg.pinv(Sigma)
    det = float(np.linalg.det(Sigma))
    phi = float(np.asarray(Phi).reshape(-1)[0])

    denom = math.sqrt(TWO_PI * det) if det > 0 else 0.0
    A = phi / denom if denom > 0 else 0.0
    lnA = math.log(A) if A > 0 else -1e30

    bias_tr_val = np.float32(lnA)

    general = not np.allclose(inv, np.eye(64), atol=1e-6)
    if general:
        vals, vecs = np.linalg.eigh(inv)
        vals = np.clip(vals, 0.0, None)
        W = (np.sqrt(vals)[:, None] * vecs.T)  # W^T W = inv
        Wmu = W @ mu.astype(np.float64)
        W2 = np.zeros((P, P), dtype=np.float32)
        W2[:D, :D] = W.T  # lhsT: stationary, out = lhsT.T @ rhs = W @ rhs
        W2[D:, D:] = W.T
        negmu2 = np.concatenate([-Wmu, -Wmu]).astype(np.float32)[:, None]
    else:
        W2 = None
        negmu2 = np.concatenate([-mu, -mu]).astype(np.float32)[:, None]

    negh2 = np.zeros((P, 2), dtype=np.float32)
    negh2[:D, 0] = -0.5
    negh2[D:, 1] = -0.5

    consts = {
        "ident": np.eye(P, dtype=np.float32),
        "negmu2": np.ascontiguousarray(negmu2, dtype=np.float32),
        "bias_tr": np.full((P, 1), bias_tr_val, dtype=np.float32),
        "negh2": negh2,  # cast to bf16 at feed time
    }
    return consts, general, W2


_NC_CACHE = {}

# Knobs for test harnesses (not used by graders): set TRACE=True before a
# call to profile; LAST_EXEC_NS / LAST_RESULTS are populated per call.
TRACE = False
LAST_EXEC_NS = None
LAST_RESULTS = None


def kernel(samples, Phi, mu, Sigma):
    global LAST_EXEC_NS, LAST_RESULTS
    from concourse.bass_utils import run_bass_kernel_spmd

    samples = np.ascontiguousarray(samples, dtype=np.float32)
    N = samples.shape[0]
    n_cores = 8
    n_per_core = N // n_cores
    assert n_per_core * n_cores == N

    consts, general, W2 = _host_prep(Phi, mu, Sigma)

    n_tiles = n_per_core // TILE_SAMPLES
    tiles_per_batch = min(BATCH_TILES, n_tiles)

    key = (n_per_core, tiles_per_batch, general)
    if key not in _NC_CACHE:
        _NC_CACHE[key] = _build_nc(n_per_core, tiles_per_batch, general)
    nc = _NC_CACHE[key]

    import ml_dtypes

    base = dict(consts)
    base["negh2"] = base["negh2"].astype(ml_dtypes.bfloat16)
    base["ident"] = base["ident"].astype(ml_dtypes.bfloat16)
    if general:
        base["w2"] = W2.astype(ml_dtypes.bfloat16)

    in_maps = []
    for c in range(n_cores):
        m = dict(base)
        m["x"] = samples[c * n_per_core:(c + 1) * n_per_core]
        in_maps.append(m)

    res = run_bass_kernel_spmd(
        nc, in_maps, core_ids=list(range(n_cores)), trace=TRACE
    )
    LAST_EXEC_NS = res.exec_time_ns
    LAST_RESULTS = res
    outs = [r["out"] for r in res.results]
    return np.concatenate(outs, axis=0)


# revision 40
# speedup vs baseline: 1.0087x; 1.0087x over previous
"""Trainium2 Bass kernel for nn_Mixture: out = -log(Phi*exp(-0.5*q)/sqrt(2*pi*det(Sigma)) + eps)
with q_n = (x_n - mu)^T pinv(Sigma) (x_n - mu), x: [N, 64] f32.

Strategy (pure data parallel over 8 cores):
  - Host: tiny O(D^3) precompute: pinv/det/eigh of Sigma (64x64), constants.
  - Device per core (N/8 = 131072 samples):
      * contiguous DMA loads [128, 8192] f32 (128 consecutive samples per partition)
      * PE transpose 128x128 chunks -> PSUM (dims land on partitions)
      * ACT: sq = Square(x + (-mu))  (mu folded in as per-partition bias), bf16 out
      * PE: lhsT = block "-0.5*ones" [128, 2] -> QP[2u+g, :] = -0.5 * q  (64->1 reduce)
      * ACT: Softplus(QP + (ln(A) - ln(eps)))   [exact: -log(e^t+eps) = -ln(eps)-softplus(t-ln(eps))]
      * DVE: out = C - softplus(...)
      * PE re-transpose of the small q-grid (interleaved out AP) to restore sample order
      * contiguous DMA out
  If pinv(Sigma) != I: an extra PE matmul with W2 = blockdiag(W, W), W^T W = pinv(Sigma),
  is inserted between transpose and Square (slower but correct for any PSD Sigma).
"""

import math
import os
import sys

import numpy as np

sys.path.insert(0, "/opt/trn_rl_repo")

import concourse.bass as bass  # noqa: E402
import concourse.bacc as bacc_mod  # noqa: E402
import concourse.mybir as mybir  # noqa: E402
from concourse.tile import TileContext  # noqa: E402

TWO_PI = 2.0 * np.pi
EPS = 1e-8

# Force all our ACT funcs (Square/Exp/Ln, + Copy/Identity for the general
# path) into ONE table set so the scalar engine never swaps table sets
# mid-kernel (~2.7us per swap). We keep the set list order identical (ids
# are positional indices into act_info.json) and just remove our funcs
# from every other set so the chooser must pick the shared one.
_ACT_PATCHED = False


def _patch_act_tables():
    global _ACT_PATCHED
    if _ACT_PATCHED:
        return
    import functools

    import concourse.hw_specs as hw_specs
    import concourse.bacc as _bacc

    orig = hw_specs.get_activation_tables
    AF = mybir.ActivationFunctionType
    need = {AF.Square, AF.Exp, AF.Ln, AF.Identity, AF.Copy, AF.MemsetZero}
    keep = "natural_log_exp_and_others"

    @functools.cache
    def patched(module_arch):
        t = dict(orig(module_arch))
        if keep in t and need <= t[keep]:
            t = {
                name: (funcs if name == keep else funcs - need)
                for name, funcs in t.items()
            }
        return t

    hw_specs.get_activation_tables = patched
    _bacc.get_activation_tables = patched
    _ACT_PATCHED = True

# Optional experiment: slim the TileContext end-of-kernel drain/barrier
# (saves a few us; the kernel preamble re-clears semaphores each launch).
_TAIL_PATCHED = False


def _patch_tail():
    global _TAIL_PATCHED
    if _TAIL_PATCHED:
        return
    from concourse import tile as tile_mod
    from concourse.vector_clock import ScopedClock

    def slim_drain_and_barrier(self, tick_clock, wait_clock):
        drain_inst = self.nc.sync.drain()
        wait_clock.add_sem_waits(
            drain_inst.ins, ScopedClock({None: tick_clock.global_clock})
        )
        self.nc.all_engine_barrier()
        popped = self.nc._tile_sem_poison_stack.pop()
        assert popped is self._sem_poison
        self.nc.clear_and_free_semaphores(
            list(self.sems.allocated().values())
        )

    tile_mod.TileContext._drain_and_barrier = slim_drain_and_barrier
    _TAIL_PATCHED = True


P = 128          # partitions
D = 64           # sample dim
CHUNK = 128      # transpose chunk cols
MG = 8           # m-groups per macro tile
TILE_COLS = MG * 512           # 4096 f32 per partition per macro tile
TILE_SAMPLES = P * TILE_COLS // D  # 8192 samples per macro tile
BATCH_TILES = 8  # macro tiles per q-grid batch (grid 128x512)


def _build_nc(n_per_core: int, tiles_per_batch: int, general: bool):
    """Build the SPMD per-core Bass program."""
    _patch_act_tables()
    if os.environ.get("KERNEL_SLIM_TAIL"):
        _patch_tail()
    assert n_per_core % TILE_SAMPLES == 0
    n_tiles = n_per_core // TILE_SAMPLES
    assert n_tiles % tiles_per_batch == 0
    n_batches = n_tiles // tiles_per_batch
    qp_rows = 2 * tiles_per_batch * MG
    assert qp_rows <= 128

    f32 = mybir.dt.float32
    bf16 = mybir.dt.bfloat16
    AF = mybir.ActivationFunctionType

    nc = bacc_mod.Bacc("TRN2")
    x = nc.dram_tensor("x", [n_per_core, D], f32, kind="ExternalInput")
    ident = nc.dram_tensor("ident", [P, P], bf16, kind="ExternalInput")
    negmu2 = nc.dram_tensor("negmu2", [P, 1], f32, kind="ExternalInput")
    bias_tr = nc.dram_tensor("bias_tr", [P, 1], f32, kind="ExternalInput")
    negh2 = nc.dram_tensor("negh2", [P, 2], bf16, kind="ExternalInput")
    if general:
        w2 = nc.dram_tensor("w2", [P, P], bf16, kind="ExternalInput")
    out = nc.dram_tensor("out", [n_per_core], f32, kind="ExternalOutput")

    jw = TILE_COLS // D  # samples per partition per tile (q-grid cols per tl)
    # [tile, 128, TILE_COLS] view: partition p holds jw consecutive samples
    x_v = x.rearrange("(t p k) d -> t p (k d)", p=P, k=jw)
    # output view: sample = b*BS + tl*TILE_SAMPLES + r*jw + j
    out_v = out.rearrange(
        "(b t r j) -> b r t j", t=tiles_per_batch, r=P, j=jw
    )
    grid_cols = tiles_per_batch * jw

    C = -math.log(EPS)

    # m-groups per PSUM tile: 4 (fast; [128,2048] bf16 = 2 banks) keeps the
    # ACT squares at FD=2048; general path needs PSUM room for z
    m2w = 2 if general else 4
    with TileContext(nc) as tc:
        with (
            tc.tile_pool(name="consts", bufs=1) as cpool,
            tc.tile_pool(name="xin", bufs=5) as in_pool,
            tc.tile_pool(name="xinb", bufs=8) as inb_pool,
            tc.tile_pool(name="tp", bufs=2 if general else 3, space="PSUM") as tp_pool,
            tc.tile_pool(name="z", bufs=1, space="PSUM") as z_pool,
            tc.tile_pool(name="sq", bufs=6) as sq_pool,
            tc.tile_pool(name="qp", bufs=2, space="PSUM") as qp_pool,
            tc.tile_pool(name="post", bufs=2) as post_pool,
        ):
            ident_t = cpool.tile([P, P], bf16)
            nc.sync.dma_start(out=ident_t, in_=ident[:, :])
            negmu2_t = cpool.tile([P, 1], f32)
            nc.sync.dma_start(out=negmu2_t, in_=negmu2[:, :])
            bias_tr_t = cpool.tile([P, 1], f32)
            nc.sync.dma_start(out=bias_tr_t, in_=bias_tr[:, :])
            negh2_t = cpool.tile([P, 2], bf16)
            nc.sync.dma_start(out=negh2_t, in_=negh2[:, :])
            eps_t = cpool.tile([P, 1], f32)
            nc.vector.memset(eps_t, float(EPS))
            if general:
                w2_t = cpool.tile([P, P], bf16)
                nc.sync.dma_start(out=w2_t, in_=w2[:, :])

            for b in range(n_batches):
                qp = qp_pool.tile([P, grid_cols], f32, name=f"qp_{b}", tag="qp")
                for tl in range(tiles_per_batch):
                    t = b * tiles_per_batch + tl
                    tin = in_pool.tile([P, TILE_COLS], f32, tag="tin")
                    tinb = inb_pool.tile([P, TILE_COLS], bf16, tag="tinb")
                    # cast to bf16 on the (otherwise idle) vector engine:
                    # halves the PE transpose cost and makes the stationary
                    # loads FWL-eligible. Sliced DMAs + casts pipeline the
                    # first tile so transposes start after ~512KB.
                    if t == 0:
                        n_sl = 4
                        scols = TILE_COLS // n_sl
                        for si in range(n_sl):
                            sl = slice(si * scols, (si + 1) * scols)
                            nc.sync.dma_start(
                                out=tin[:, sl], in_=x_v[t][:, sl]
                            )
                            nc.vector.tensor_copy(tinb[:, sl], tin[:, sl])
                    else:
                        nc.sync.dma_start(out=tin, in_=x_v[t])
                        nc.vector.tensor_copy(tinb, tin)
                    for m2 in range(MG // m2w):
                        # m2w m-groups share one PSUM tile so the ACT
                        # square runs at a large free dim
                        tp = tp_pool.tile([P, 512 * m2w], bf16, tag="tp")
                        for mh in range(m2w):
                            m = m2 * m2w + mh
                            for c in range(4):
                                nc.tensor.transpose(
                                    tp[:, mh * 512 + c * CHUNK:
                                       mh * 512 + (c + 1) * CHUNK],
                                    tinb[:, m * 512 + c * CHUNK:
                                         m * 512 + (c + 1) * CHUNK],
                                    ident_t,
                                )
                        sq = sq_pool.tile([P, 512 * m2w], bf16, tag="sq")
                        if general:
                            # z = W2 @ x, then sq = Square(z - W2@mu2)
                            xt = sq_pool.tile([P, 512 * m2w], bf16, tag="xt")
                            nc.scalar.activation(xt, tp, AF.Copy, scale=1.0)
                            z = z_pool.tile([P, 512 * m2w], f32, tag="z")
                            nc.tensor.matmul(
                                z, w2_t, xt, start=True, stop=True
                            )
                            nc.scalar.activation(
                                sq, z, AF.Square, bias=negmu2_t, scale=1.0
                            )
                        else:
                            nc.scalar.activation(
                                sq, tp, AF.Square, bias=negmu2_t, scale=1.0
                            )
                        # reduce 64 dims -> q: sq chunk is the STATIONARY
                        # (samples land on output partitions), -0.5 block
                        # matrix is the moving; output col-pair lands at
                        # free offset tl*jw + m*8 + c*2 in the q grid.
                        for mh in range(m2w):
                            m = m2 * m2w + mh
                            for c in range(4):
                                base = tl * jw + m * 8 + c * 2
                                nc.tensor.matmul(
                                    qp[:, base:base + 2],
                                    sq[:, mh * 512 + c * CHUNK:
                                       mh * 512 + (c + 1) * CHUNK],
                                    negh2_t,
                                    start=True, stop=True,
                                )
                # post-process the q grid: out = -log(exp(qp + lnA) + eps)
                s = post_pool.tile([P, grid_cols], f32, tag="s")
                nc.scalar.activation(
                    s, qp, AF.Exp, bias=bias_tr_t, scale=1.0,
                )
                s2 = post_pool.tile([P, grid_cols], f32, tag="s2")
                nc.scalar.activation(
                    s2, s, AF.Ln, bias=eps_t, scale=1.0,
                )
                o = post_pool.tile([P, grid_cols], f32, tag="o")
                nc.vector.tensor_scalar(
                    o, s2, -1.0, 0.0, mybir.AluOpType.mult, mybir.AluOpType.add
                )
                nc.sync.dma_start(
                    out=out_v[b],
                    in_=o.rearrange("p (t j) -> p t j", t=tiles_per_batch),
                )
    nc.finalize()
    return nc


def _host_prep(Phi, mu, Sigma):
    """Host-side tiny precompute. Returns (consts dict, general, W2)."""
    Sigma = np.asarray(Sigma, dtype=np.float64)
    mu = np.asarray(mu, dtype=np.float32)
    inv = np.linalg.pinv(Sigma)
    det = float(np.linalg.det(Sigma))
    phi = float(np.asarray(Phi).reshape(-1)[0])

    denom = math.sqrt(TWO_PI * det) if det > 0 else 0.0
    A = phi / denom if denom > 0 else 0.0
    lnA = math.log(A) if A > 0 else -1e30

    bias_tr_val = np.float32(lnA)

    general = not np.allclose(inv, np.eye(64), atol=1e-6)
    if general:
        vals, vecs = np.linalg.eigh(inv)
        vals = np.clip(vals, 0.0, None)
        W = (np.sqrt(vals)[:, None] * vecs.T)  # W^T W = inv
        Wmu = W @ mu.astype(np.float64)
        W2 = np.zeros((P, P), dtype=np.float32)
        W2[:D, :D] = W.T  # lhsT: stationary, out = lhsT.T @ rhs = W @ rhs
        W2[D:, D:] = W.T
        negmu2 = np.concatenate([-Wmu, -Wmu]).astype(np.float32)[:, None]
    else:
        W2 = None
        negmu2 = np.concatenate([-mu, -mu]).astype(np.float32)[:, None]

    negh2 = np.zeros((P, 2), dtype=np.float32)
    negh2[:D, 0] = -0.5
    negh2[D:, 1] = -0.5

    consts = {
        "ident": np.eye(P, dtype=np.float32),
        "negmu2": np.ascontiguousarray(negmu2, dtype=np.float32),
        "bias_tr": np.full((P, 1), bias_tr_val, dtype=np.float32),
        "negh2": negh2,  # cast to bf16 at feed time
    }
    return consts, general, W2


_NC_CACHE = {}

# Knobs for test harnesses (not used by graders): set TRACE=True before a
# call to profile; LAST_EXEC_NS / LAST_RESULTS are populated per call.
TRACE = False
LAST_EXEC_NS = None
LAST_RESULTS = None


def kernel(samples, Phi, mu, Sigma):
    global LAST_EXEC_NS, LAST_RESULTS
    from concourse.bass_utils import run_bass_kernel_spmd

    samples = np.ascontiguousarray(samples, dtype=np.float32)
    N = samples.shape[0]
    n_cores = 8
    n_per_core = N // n_cores
    assert n_per_core * n_cores == N

    consts, general, W2 = _host_prep(Phi, mu, Sigma)

    n_tiles = n_per_core // TILE_SAMPLES
    tiles_per_batch = min(BATCH_TILES, n_tiles)

    key = (n_per_core, tiles_per_batch, general)
    if key not in _NC_CACHE:
        _NC_CACHE[key] = _build_nc(n_per_core, tiles_per_batch, general)
    nc = _NC_CACHE[key]

    import ml_dtypes

    base = dict(consts)
    base["negh2"] = base["negh2"].astype(ml_dtypes.bfloat16)
    base["ident"] = base["ident"].astype(ml_dtypes.bfloat16)
    if general:
        base["w2"] = W2.astype(ml_dtypes.bfloat16)

    in_maps = []
    for c in range(n_cores):
        m = dict(base)
        m["x"] = samples[c * n_per_core:(c + 1) * n_per_core]
        in_maps.append(m)

    res = run_bass_kernel_spmd(
        nc, in_maps, core_ids=list(range(n_cores)), trace=TRACE
    )
    LAST_EXEC_NS = res.exec_time_ns
    LAST_RESULTS = res
    outs = [r["out"] for r in res.results]
    return np.concatenate(outs, axis=0)


# revision 41
# speedup vs baseline: 1.1417x; 1.1319x over previous
"""Trainium2 Bass kernel for nn_Mixture: out = -log(Phi*exp(-0.5*q)/sqrt(2*pi*det(Sigma)) + eps)
with q_n = (x_n - mu)^T pinv(Sigma) (x_n - mu), x: [N, 64] f32.

Strategy (pure data parallel over 8 cores):
  - Host: tiny O(D^3) precompute: pinv/det/eigh of Sigma (64x64), constants.
  - Device per core (N/8 = 131072 samples):
      * contiguous DMA loads [128, 8192] f32 (128 consecutive samples per partition)
      * PE transpose 128x128 chunks -> PSUM (dims land on partitions)
      * ACT: sq = Square(x + (-mu))  (mu folded in as per-partition bias), bf16 out
      * PE: lhsT = block "-0.5*ones" [128, 2] -> QP[2u+g, :] = -0.5 * q  (64->1 reduce)
      * ACT: Softplus(QP + (ln(A) - ln(eps)))   [exact: -log(e^t+eps) = -ln(eps)-softplus(t-ln(eps))]
      * DVE: out = C - softplus(...)
      * PE re-transpose of the small q-grid (interleaved out AP) to restore sample order
      * contiguous DMA out
  If pinv(Sigma) != I: an extra PE matmul with W2 = blockdiag(W, W), W^T W = pinv(Sigma),
  is inserted between transpose and Square (slower but correct for any PSD Sigma).
"""

import math
import os
import sys

import numpy as np

sys.path.insert(0, "/opt/trn_rl_repo")

import concourse.bass as bass  # noqa: E402
import concourse.bacc as bacc_mod  # noqa: E402
import concourse.mybir as mybir  # noqa: E402
from concourse.tile import TileContext  # noqa: E402

TWO_PI = 2.0 * np.pi
EPS = 1e-8

# Force all our ACT funcs (Square/Exp/Ln, + Copy/Identity for the general
# path) into ONE table set so the scalar engine never swaps table sets
# mid-kernel (~2.7us per swap). We keep the set list order identical (ids
# are positional indices into act_info.json) and just remove our funcs
# from every other set so the chooser must pick the shared one.
_ACT_PATCHED = False


def _patch_act_tables():
    global _ACT_PATCHED
    if _ACT_PATCHED:
        return
    import functools

    import concourse.hw_specs as hw_specs
    import concourse.bacc as _bacc

    orig = hw_specs.get_activation_tables
    AF = mybir.ActivationFunctionType
    need = {AF.Square, AF.Exp, AF.Ln, AF.Identity, AF.Copy, AF.MemsetZero}
    keep = "natural_log_exp_and_others"

    @functools.cache
    def patched(module_arch):
        t = dict(orig(module_arch))
        if keep in t and need <= t[keep]:
            t = {
                name: (funcs if name == keep else funcs - need)
                for name, funcs in t.items()
            }
        return t

    hw_specs.get_activation_tables = patched
    _bacc.get_activation_tables = patched
    _ACT_PATCHED = True

# Optional experiment: slim the TileContext end-of-kernel drain/barrier
# (saves a few us; the kernel preamble re-clears semaphores each launch).
_TAIL_PATCHED = False


def _patch_tail():
    global _TAIL_PATCHED
    if _TAIL_PATCHED:
        return
    from concourse import tile as tile_mod
    from concourse.vector_clock import ScopedClock

    def slim_drain_and_barrier(self, tick_clock, wait_clock):
        drain_inst = self.nc.sync.drain()
        wait_clock.add_sem_waits(
            drain_inst.ins, ScopedClock({None: tick_clock.global_clock})
        )
        self.nc.all_engine_barrier()
        popped = self.nc._tile_sem_poison_stack.pop()
        assert popped is self._sem_poison
        self.nc.clear_and_free_semaphores(
            list(self.sems.allocated().values())
        )

    tile_mod.TileContext._drain_and_barrier = slim_drain_and_barrier
    _TAIL_PATCHED = True


P = 128          # partitions
D = 64           # sample dim
CHUNK = 128      # transpose chunk cols
MG = 8           # m-groups per macro tile
TILE_COLS = MG * 512           # 4096 f32 per partition per macro tile
TILE_SAMPLES = P * TILE_COLS // D  # 8192 samples per macro tile
BATCH_TILES = 8  # macro tiles per q-grid batch (grid 128x512)


def _build_nc(n_per_core: int, tiles_per_batch: int, general: bool):
    """Build the SPMD per-core Bass program."""
    _patch_act_tables()
    if os.environ.get("KERNEL_SLIM_TAIL"):
        _patch_tail()
    assert n_per_core % TILE_SAMPLES == 0
    n_tiles = n_per_core // TILE_SAMPLES
    assert n_tiles % tiles_per_batch == 0
    n_batches = n_tiles // tiles_per_batch
    qp_rows = 2 * tiles_per_batch * MG
    assert qp_rows <= 128

    f32 = mybir.dt.float32
    bf16 = mybir.dt.bfloat16
    AF = mybir.ActivationFunctionType

    nc = bacc_mod.Bacc("TRN2")
    x = nc.dram_tensor("x", [n_per_core, D], f32, kind="ExternalInput")
    ident = nc.dram_tensor("ident", [P, P], bf16, kind="ExternalInput")
    negmu2 = nc.dram_tensor("negmu2", [P, 1], f32, kind="ExternalInput")
    bias_tr = nc.dram_tensor("bias_tr", [P, 1], f32, kind="ExternalInput")
    negh2 = nc.dram_tensor("negh2", [P, 2], bf16, kind="ExternalInput")
    if general:
        w2 = nc.dram_tensor("w2", [P, P], bf16, kind="ExternalInput")
    out = nc.dram_tensor("out", [n_per_core], f32, kind="ExternalOutput")

    jw = TILE_COLS // D  # samples per partition per tile (q-grid cols per tl)
    # [tile, 128, TILE_COLS] view: partition p holds jw consecutive samples
    x_v = x.rearrange("(t p k) d -> t p (k d)", p=P, k=jw)
    # output view: sample = b*BS + tl*TILE_SAMPLES + r*jw + j
    out_v = out.rearrange(
        "(b t r j) -> b r t j", t=tiles_per_batch, r=P, j=jw
    )
    grid_cols = tiles_per_batch * jw

    C = -math.log(EPS)

    # m-groups per PSUM tile: 4 (fast; [128,2048] bf16 = 2 banks) keeps the
    # ACT squares at FD=2048; general path needs PSUM room for z
    m2w = 2 if general else 4
    with TileContext(nc) as tc:
        with (
            tc.tile_pool(name="consts", bufs=1) as cpool,
            tc.tile_pool(name="xin", bufs=5) as in_pool,
            tc.tile_pool(name="xinb", bufs=8) as inb_pool,
            tc.tile_pool(name="tp", bufs=2 if general else 3, space="PSUM") as tp_pool,
            tc.tile_pool(name="z", bufs=1, space="PSUM") as z_pool,
            tc.tile_pool(name="sq", bufs=6) as sq_pool,
            tc.tile_pool(name="qp", bufs=2, space="PSUM") as qp_pool,
            tc.tile_pool(name="post", bufs=2) as post_pool,
        ):
            ident_t = cpool.tile([P, P], bf16)
            nc.sync.dma_start(out=ident_t, in_=ident[:, :])
            negmu2_t = cpool.tile([P, 1], f32)
            nc.sync.dma_start(out=negmu2_t, in_=negmu2[:, :])
            bias_tr_t = cpool.tile([P, 1], f32)
            nc.sync.dma_start(out=bias_tr_t, in_=bias_tr[:, :])
            negh2_t = cpool.tile([P, 2], bf16)
            nc.sync.dma_start(out=negh2_t, in_=negh2[:, :])
            eps_t = cpool.tile([P, 1], f32)
            nc.vector.memset(eps_t, float(EPS))
            if general:
                w2_t = cpool.tile([P, P], bf16)
                nc.sync.dma_start(out=w2_t, in_=w2[:, :])

            for b in range(n_batches):
                qp = qp_pool.tile([P, grid_cols], f32, name=f"qp_{b}", tag="qp")
                for tl in range(tiles_per_batch):
                    t = b * tiles_per_batch + tl
                    # SWDGE DMA casts f32->bf16 in flight: no on-chip cast
                    # pass, half the SBUF write traffic. bf16 halves the PE
                    # transpose cost and makes stationary loads FWL-eligible.
                    tinb = inb_pool.tile([P, TILE_COLS], bf16, tag="tinb")
                    if t == 0:
                        scols = TILE_COLS // 4
                        for si in range(4):
                            sl = slice(si * scols, (si + 1) * scols)
                            nc.gpsimd.dma_start(
                                out=tinb[:, sl], in_=x_v[t][:, sl]
                            )
                    else:
                        nc.gpsimd.dma_start(out=tinb, in_=x_v[t])
                    for m2 in range(MG // m2w):
                        # m2w m-groups share one PSUM tile so the ACT
                        # square runs at a large free dim
                        tp = tp_pool.tile([P, 512 * m2w], bf16, tag="tp")
                        for mh in range(m2w):
                            m = m2 * m2w + mh
                            for c in range(4):
                                nc.tensor.transpose(
                                    tp[:, mh * 512 + c * CHUNK:
                                       mh * 512 + (c + 1) * CHUNK],
                                    tinb[:, m * 512 + c * CHUNK:
                                         m * 512 + (c + 1) * CHUNK],
                                    ident_t,
                                )
                        sq = sq_pool.tile([P, 512 * m2w], bf16, tag="sq")
                        if general:
                            # z = W2 @ x, then sq = Square(z - W2@mu2)
                            xt = sq_pool.tile([P, 512 * m2w], bf16, tag="xt")
                            nc.scalar.activation(xt, tp, AF.Copy, scale=1.0)
                            z = z_pool.tile([P, 512 * m2w], f32, tag="z")
                            nc.tensor.matmul(
                                z, w2_t, xt, start=True, stop=True
                            )
                            nc.scalar.activation(
                                sq, z, AF.Square, bias=negmu2_t, scale=1.0
                            )
                        else:
                            nc.scalar.activation(
                                sq, tp, AF.Square, bias=negmu2_t, scale=1.0
                            )
                        # reduce 64 dims -> q: sq chunk is the STATIONARY
                        # (samples land on output partitions), -0.5 block
                        # matrix is the moving; output col-pair lands at
                        # free offset tl*jw + m*8 + c*2 in the q grid.
                        for mh in range(m2w):
                            m = m2 * m2w + mh
                            for c in range(4):
                                base = tl * jw + m * 8 + c * 2
                                nc.tensor.matmul(
                                    qp[:, base:base + 2],
                                    sq[:, mh * 512 + c * CHUNK:
                                       mh * 512 + (c + 1) * CHUNK],
                                    negh2_t,
                                    start=True, stop=True,
                                )
                # post-process the q grid: out = -log(exp(qp + lnA) + eps)
                s = post_pool.tile([P, grid_cols], f32, tag="s")
                nc.scalar.activation(
                    s, qp, AF.Exp, bias=bias_tr_t, scale=1.0,
                )
                s2 = post_pool.tile([P, grid_cols], f32, tag="s2")
                nc.scalar.activation(
                    s2, s, AF.Ln, bias=eps_t, scale=1.0,
                )
                o = post_pool.tile([P, grid_cols], f32, tag="o")
                nc.vector.tensor_scalar(
                    o, s2, -1.0, 0.0, mybir.AluOpType.mult, mybir.AluOpType.add
                )
                nc.sync.dma_start(
                    out=out_v[b],
                    in_=o.rearrange("p (t j) -> p t j", t=tiles_per_batch),
                )
    nc.finalize()
    return nc


def _host_prep(Phi, mu, Sigma):
    """Host-side tiny precompute. Returns (consts dict, general, W2)."""
    Sigma = np.asarray(Sigma, dtype=np.float64)
    mu = np.asarray(mu, dtype=np.float32)
    inv = np.linalg.pinv(Sigma)
    det = float(np.linalg.det(Sigma))
    phi = float(np.asarray(Phi).reshape(-1)[0])

    denom = math.sqrt(TWO_PI * det) if det > 0 else 0.0
    A = phi / denom if denom > 0 else 0.0
    lnA = math.log(A) if A > 0 else -1e30

    bias_tr_val = np.float32(lnA)

    general = not np.allclose(inv, np.eye(64), atol=1e-6)
    if general:
        vals, vecs = np.linalg.eigh(inv)
        vals = np.clip(vals, 0.0, None)
        W = (np.sqrt(vals)[:, None] * vecs.T)  # W^T W = inv
        Wmu = W @ mu.astype(np.float64)
        W2 = np.zeros((P, P), dtype=np.float32)
        W2[:D, :D] = W.T  # lhsT: stationary, out = lhsT.T @ rhs = W @ rhs
        W2[D:, D:] = W.T
        negmu2 = np.concatenate([-Wmu, -Wmu]).astype(np.float32)[:, None]
    else:
        W2 = None
        negmu2 = np.concatenate([-mu, -mu]).astype(np.float32)[:, None]

    negh2 = np.zeros((P, 2), dtype=np.float32)
    negh2[:D, 0] = -0.5
    negh2[D:, 1] = -0.5

    consts = {
        "ident": np.eye(P, dtype=np.float32),
        "negmu2": np.ascontiguousarray(negmu2, dtype=np.float32),
        "bias_tr": np.full((P, 1), bias_tr_val, dtype=np.float32),
        "negh2": negh2,  # cast to bf16 at feed time
    }
    return consts, general, W2


_NC_CACHE = {}

# Knobs for test harnesses (not used by graders): set TRACE=True before a
# call to profile; LAST_EXEC_NS / LAST_RESULTS are populated per call.
TRACE = False
LAST_EXEC_NS = None
LAST_RESULTS = None


def kernel(samples, Phi, mu, Sigma):
    global LAST_EXEC_NS, LAST_RESULTS
    from concourse.bass_utils import run_bass_kernel_spmd

    samples = np.ascontiguousarray(samples, dtype=np.float32)
    N = samples.shape[0]
    n_cores = 8
    n_per_core = N // n_cores
    assert n_per_core * n_cores == N

    consts, general, W2 = _host_prep(Phi, mu, Sigma)

    n_tiles = n_per_core // TILE_SAMPLES
    tiles_per_batch = min(BATCH_TILES, n_tiles)

    key = (n_per_core, tiles_per_batch, general)
    if key not in _NC_CACHE:
        _NC_CACHE[key] = _build_nc(n_per_core, tiles_per_batch, general)
    nc = _NC_CACHE[key]

    import ml_dtypes

    base = dict(consts)
    base["negh2"] = base["negh2"].astype(ml_dtypes.bfloat16)
    base["ident"] = base["ident"].astype(ml_dtypes.bfloat16)
    if general:
        base["w2"] = W2.astype(ml_dtypes.bfloat16)

    in_maps = []
    for c in range(n_cores):
        m = dict(base)
        m["x"] = samples[c * n_per_core:(c + 1) * n_per_core]
        in_maps.append(m)

    res = run_bass_kernel_spmd(
        nc, in_maps, core_ids=list(range(n_cores)), trace=TRACE
    )
    LAST_EXEC_NS = res.exec_time_ns
    LAST_RESULTS = res
    outs = [r["out"] for r in res.results]
    return np.concatenate(outs, axis=0)


# revision 44
# speedup vs baseline: 1.1477x; 1.0052x over previous
"""Trainium2 Bass kernel for nn_Mixture: out = -log(Phi*exp(-0.5*q)/sqrt(2*pi*det(Sigma)) + eps)
with q_n = (x_n - mu)^T pinv(Sigma) (x_n - mu), x: [N, 64] f32.

Strategy (pure data parallel over 8 cores):
  - Host: tiny O(D^3) precompute: pinv/det/eigh of Sigma (64x64), constants.
  - Device per core (N/8 = 131072 samples):
      * contiguous DMA loads [128, 8192] f32 (128 consecutive samples per partition)
      * PE transpose 128x128 chunks -> PSUM (dims land on partitions)
      * ACT: sq = Square(x + (-mu))  (mu folded in as per-partition bias), bf16 out
      * PE: lhsT = block "-0.5*ones" [128, 2] -> QP[2u+g, :] = -0.5 * q  (64->1 reduce)
      * ACT: Softplus(QP + (ln(A) - ln(eps)))   [exact: -log(e^t+eps) = -ln(eps)-softplus(t-ln(eps))]
      * DVE: out = C - softplus(...)
      * PE re-transpose of the small q-grid (interleaved out AP) to restore sample order
      * contiguous DMA out
  If pinv(Sigma) != I: an extra PE matmul with W2 = blockdiag(W, W), W^T W = pinv(Sigma),
  is inserted between transpose and Square (slower but correct for any PSD Sigma).
"""

import math
import os
import sys

import numpy as np

sys.path.insert(0, "/opt/trn_rl_repo")

import concourse.bass as bass  # noqa: E402
import concourse.bacc as bacc_mod  # noqa: E402
import concourse.mybir as mybir  # noqa: E402
from concourse.tile import TileContext  # noqa: E402

TWO_PI = 2.0 * np.pi
EPS = 1e-8

# Force all our ACT funcs (Square/Exp/Ln, + Copy/Identity for the general
# path) into ONE table set so the scalar engine never swaps table sets
# mid-kernel (~2.7us per swap). We keep the set list order identical (ids
# are positional indices into act_info.json) and just remove our funcs
# from every other set so the chooser must pick the shared one.
_ACT_PATCHED = False


def _patch_act_tables():
    global _ACT_PATCHED
    if _ACT_PATCHED:
        return
    import functools

    import concourse.hw_specs as hw_specs
    import concourse.bacc as _bacc

    orig = hw_specs.get_activation_tables
    AF = mybir.ActivationFunctionType
    need = {AF.Square, AF.Exp, AF.Ln, AF.Identity, AF.Copy, AF.MemsetZero}
    keep = "natural_log_exp_and_others"

    @functools.cache
    def patched(module_arch):
        t = dict(orig(module_arch))
        if keep in t and need <= t[keep]:
            t = {
                name: (funcs if name == keep else funcs - need)
                for name, funcs in t.items()
            }
        return t

    hw_specs.get_activation_tables = patched
    _bacc.get_activation_tables = patched
    _ACT_PATCHED = True

# Optional experiment: slim the TileContext end-of-kernel drain/barrier
# (saves a few us; the kernel preamble re-clears semaphores each launch).
_TAIL_PATCHED = False


def _patch_tail():
    global _TAIL_PATCHED
    if _TAIL_PATCHED:
        return
    from concourse import tile as tile_mod
    from concourse.vector_clock import ScopedClock

    def slim_drain_and_barrier(self, tick_clock, wait_clock):
        drain_inst = self.nc.sync.drain()
        wait_clock.add_sem_waits(
            drain_inst.ins, ScopedClock({None: tick_clock.global_clock})
        )
        self.nc.all_engine_barrier()
        popped = self.nc._tile_sem_poison_stack.pop()
        assert popped is self._sem_poison
        self.nc.clear_and_free_semaphores(
            list(self.sems.allocated().values())
        )

    tile_mod.TileContext._drain_and_barrier = slim_drain_and_barrier
    _TAIL_PATCHED = True


P = 128          # partitions
D = 64           # sample dim
CHUNK = 128      # transpose chunk cols
MG = 8           # m-groups per macro tile
TILE_COLS = MG * 512           # 4096 f32 per partition per macro tile
TILE_SAMPLES = P * TILE_COLS // D  # 8192 samples per macro tile
BATCH_TILES = 4  # macro tiles per q-grid batch (grid 128x256)


def _build_nc(n_per_core: int, tiles_per_batch: int, general: bool):
    """Build the SPMD per-core Bass program."""
    _patch_act_tables()
    if not os.environ.get("KERNEL_NO_SLIM_TAIL"):
        _patch_tail()
    assert n_per_core % TILE_SAMPLES == 0
    n_tiles = n_per_core // TILE_SAMPLES
    assert n_tiles % tiles_per_batch == 0
    n_batches = n_tiles // tiles_per_batch
    qp_rows = 2 * tiles_per_batch * MG
    assert qp_rows <= 128

    f32 = mybir.dt.float32
    bf16 = mybir.dt.bfloat16
    AF = mybir.ActivationFunctionType

    nc = bacc_mod.Bacc("TRN2")
    x = nc.dram_tensor("x", [n_per_core, D], f32, kind="ExternalInput")
    ident = nc.dram_tensor("ident", [P, P], bf16, kind="ExternalInput")
    negmu2 = nc.dram_tensor("negmu2", [P, 1], f32, kind="ExternalInput")
    bias_tr = nc.dram_tensor("bias_tr", [P, 1], f32, kind="ExternalInput")
    negh2 = nc.dram_tensor("negh2", [P, 2], bf16, kind="ExternalInput")
    if general:
        w2 = nc.dram_tensor("w2", [P, P], bf16, kind="ExternalInput")
    out = nc.dram_tensor("out", [n_per_core], f32, kind="ExternalOutput")

    jw = TILE_COLS // D  # samples per partition per tile (q-grid cols per tl)
    # [tile, 128, TILE_COLS] view: partition p holds jw consecutive samples
    x_v = x.rearrange("(t p k) d -> t p (k d)", p=P, k=jw)
    # output view: sample = b*BS + tl*TILE_SAMPLES + r*jw + j
    out_v = out.rearrange(
        "(b t r j) -> b r t j", t=tiles_per_batch, r=P, j=jw
    )
    grid_cols = tiles_per_batch * jw

    C = -math.log(EPS)

    # m-groups per PSUM tile: 4 (fast; [128,2048] bf16 = 2 banks) keeps the
    # ACT squares at FD=2048; general path needs PSUM room for z
    m2w = 2 if general else 4
    with TileContext(nc) as tc:
        with (
            tc.tile_pool(name="consts", bufs=1) as cpool,
            tc.tile_pool(name="xinb", bufs=10) as inb_pool,
            tc.tile_pool(name="tp", bufs=2 if general else 3, space="PSUM") as tp_pool,
            tc.tile_pool(name="z", bufs=1, space="PSUM") as z_pool,
            tc.tile_pool(name="sq", bufs=6) as sq_pool,
            tc.tile_pool(name="qp", bufs=2, space="PSUM") as qp_pool,
            tc.tile_pool(name="post", bufs=2) as post_pool,
        ):
            ident_t = cpool.tile([P, P], bf16)
            nc.sync.dma_start(out=ident_t, in_=ident[:, :])
            negmu2_t = cpool.tile([P, 1], f32)
            nc.sync.dma_start(out=negmu2_t, in_=negmu2[:, :])
            bias_tr_t = cpool.tile([P, 1], f32)
            nc.sync.dma_start(out=bias_tr_t, in_=bias_tr[:, :])
            negh2_t = cpool.tile([P, 2], bf16)
            nc.sync.dma_start(out=negh2_t, in_=negh2[:, :])
            eps_t = cpool.tile([P, 1], f32)
            nc.vector.memset(eps_t, float(EPS))
            if general:
                w2_t = cpool.tile([P, P], bf16)
                nc.sync.dma_start(out=w2_t, in_=w2[:, :])

            for b in range(n_batches):
                qp = qp_pool.tile([P, grid_cols], f32, name=f"qp_{b}", tag="qp")
                for tl in range(tiles_per_batch):
                    t = b * tiles_per_batch + tl
                    # SWDGE DMA casts f32->bf16 in flight: no on-chip cast
                    # pass, half the SBUF write traffic. bf16 halves the PE
                    # transpose cost and makes stationary loads FWL-eligible.
                    tinb = inb_pool.tile([P, TILE_COLS], bf16, tag="tinb")
                    if t == 0:
                        scols = TILE_COLS // 4
                        for si in range(4):
                            sl = slice(si * scols, (si + 1) * scols)
                            nc.gpsimd.dma_start(
                                out=tinb[:, sl], in_=x_v[t][:, sl]
                            )
                    else:
                        nc.gpsimd.dma_start(out=tinb, in_=x_v[t])
                    for m2 in range(MG // m2w):
                        # m2w m-groups share one PSUM tile so the ACT
                        # square runs at a large free dim
                        tp = tp_pool.tile([P, 512 * m2w], bf16, tag="tp")
                        for mh in range(m2w):
                            m = m2 * m2w + mh
                            for c in range(4):
                                nc.tensor.transpose(
                                    tp[:, mh * 512 + c * CHUNK:
                                       mh * 512 + (c + 1) * CHUNK],
                                    tinb[:, m * 512 + c * CHUNK:
                                         m * 512 + (c + 1) * CHUNK],
                                    ident_t,
                                )
                        sq = sq_pool.tile([P, 512 * m2w], bf16, tag="sq")
                        if general:
                            # z = W2 @ x, then sq = Square(z - W2@mu2)
                            xt = sq_pool.tile([P, 512 * m2w], bf16, tag="xt")
                            nc.scalar.activation(xt, tp, AF.Copy, scale=1.0)
                            z = z_pool.tile([P, 512 * m2w], f32, tag="z")
                            nc.tensor.matmul(
                                z, w2_t, xt, start=True, stop=True
                            )
                            nc.scalar.activation(
                                sq, z, AF.Square, bias=negmu2_t, scale=1.0
                            )
                        else:
                            nc.scalar.activation(
                                sq, tp, AF.Square, bias=negmu2_t, scale=1.0
                            )
                        # reduce 64 dims -> q: sq chunk is the STATIONARY
                        # (samples land on output partitions), -0.5 block
                        # matrix is the moving; output col-pair lands at
                        # free offset tl*jw + m*8 + c*2 in the q grid.
                        for mh in range(m2w):
                            m = m2 * m2w + mh
                            for c in range(4):
                                base = tl * jw + m * 8 + c * 2
                                nc.tensor.matmul(
                                    qp[:, base:base + 2],
                                    sq[:, mh * 512 + c * CHUNK:
                                       mh * 512 + (c + 1) * CHUNK],
                                    negh2_t,
                                    start=True, stop=True,
                                )
                # post-process the q grid: out = -log(exp(qp + lnA) + eps)
                s = post_pool.tile([P, grid_cols], f32, tag="s")
                nc.scalar.activation(
                    s, qp, AF.Exp, bias=bias_tr_t, scale=1.0,
                )
                s2 = post_pool.tile([P, grid_cols], f32, tag="s2")
                nc.scalar.activation(
                    s2, s, AF.Ln, bias=eps_t, scale=1.0,
                )
                o = post_pool.tile([P, grid_cols], f32, tag="o")
                nc.vector.tensor_scalar(
                    o, s2, -1.0, 0.0, mybir.AluOpType.mult, mybir.AluOpType.add
                )
                nc.sync.dma_start(
                    out=out_v[b],
                    in_=o.rearrange("p (t j) -> p t j", t=tiles_per_batch),
                )
    nc.finalize()
    return nc


def _host_prep(Phi, mu, Sigma):
    """Host-side tiny precompute. Returns (consts dict, general, W2)."""
    Sigma = np.asarray(Sigma, dtype=np.float64)
    mu = np.asarray(mu, dtype=np.float32)
    inv = np.linalg.pinv(Sigma)
    det = float(np.linalg.det(Sigma))
    phi = float(np.asarray(Phi).reshape(-1)[0])

    denom = math.sqrt(TWO_PI * det) if det > 0 else 0.0
    A = phi / denom if denom > 0 else 0.0
    lnA = math.log(A) if A > 0 else -1e30

    bias_tr_val = np.float32(lnA)

    general = not np.allclose(inv, np.eye(64), atol=1e-6)
    if general:
        vals, vecs = np.linalg.eigh(inv)
        vals = np.clip(vals, 0.0, None)
        W = (np.sqrt(vals)[:, None] * vecs.T)  # W^T W = inv
        Wmu = W @ mu.astype(np.float64)
        W2 = np.zeros((P, P), dtype=np.float32)
        W2[:D, :D] = W.T  # lhsT: stationary, out = lhsT.T @ rhs = W @ rhs
        W2[D:, D:] = W.T
        negmu2 = np.concatenate([-Wmu, -Wmu]).astype(np.float32)[:, None]
    else:
        W2 = None
        negmu2 = np.concatenate([-mu, -mu]).astype(np.float32)[:, None]

    negh2 = np.zeros((P, 2), dtype=np.float32)
    negh2[:D, 0] = -0.5
    negh2[D:, 1] = -0.5

    consts = {
        "ident": np.eye(P, dtype=np.float32),
        "negmu2": np.ascontiguousarray(negmu2, dtype=np.float32),
        "bias_tr": np.full((P, 1), bias_tr_val, dtype=np.float32),
        "negh2": negh2,  # cast to bf16 at feed time
    }
    return consts, general, W2


_NC_CACHE = {}

# Knobs for test harnesses (not used by graders): set TRACE=True before a
# call to profile; LAST_EXEC_NS / LAST_RESULTS are populated per call.
TRACE = False
LAST_EXEC_NS = None
LAST_RESULTS = None


def kernel(samples, Phi, mu, Sigma):
    global LAST_EXEC_NS, LAST_RESULTS
    from concourse.bass_utils import run_bass_kernel_spmd

    samples = np.ascontiguousarray(samples, dtype=np.float32)
    N = samples.shape[0]
    n_cores = 8
    n_per_core = N // n_cores
    assert n_per_core * n_cores == N

    consts, general, W2 = _host_prep(Phi, mu, Sigma)

    n_tiles = n_per_core // TILE_SAMPLES
    tiles_per_batch = min(BATCH_TILES, n_tiles)

    key = (n_per_core, tiles_per_batch, general)
    if key not in _NC_CACHE:
        _NC_CACHE[key] = _build_nc(n_per_core, tiles_per_batch, general)
    nc = _NC_CACHE[key]

    import ml_dtypes

    base = dict(consts)
    base["negh2"] = base["negh2"].astype(ml_dtypes.bfloat16)
    base["ident"] = base["ident"].astype(ml_dtypes.bfloat16)
    if general:
        base["w2"] = W2.astype(ml_dtypes.bfloat16)

    in_maps = []
    for c in range(n_cores):
        m = dict(base)
        m["x"] = samples[c * n_per_core:(c + 1) * n_per_core]
        in_maps.append(m)

    res = run_bass_kernel_spmd(
        nc, in_maps, core_ids=list(range(n_cores)), trace=TRACE
    )
    LAST_EXEC_NS = res.exec_time_ns
    LAST_RESULTS = res
    outs = [r["out"] for r in res.results]
    return np.concatenate(outs, axis=0)
